# revision 16
# baseline (speedup 1.0000x reference)
"""MoE layer (B=2,T=1024,D=2048,F=768,E=16,K=2) on 8 NeuronCores.

Expert-parallel: core c owns experts {2c, 2c+1}. Host computes the router
(scores -> softmax -> top-2 -> renormalize; ~0.3% of total FLOPs), gathers
each expert's tokens into fixed-capacity transposed buffers, and the device
kernel runs the sparse SwiGLU FFN (gate/up/down matmuls) in bf16 with f32
PSUM accumulation.

Layout tricks:
- tokens staged transposed (xgt [D, C]) so gate/up produce hT [F, C] directly
  in the lhsT layout the down projection wants -- zero on-device transposes.
- the routing weight is pre-multiplied into the up-projection copy of the
  tokens (h = silu(g) * (u * cw)), so outputs come out pre-scaled.
- down projection emits yT [D, C]: matmul free dim = C exactly, no
  padded-token chunk rounding; host transposes during the scatter-add.
- weights host-pre-tiled so every DMA is >=4KB-contiguous per partition;
  token DMAs ride the scalar HW-DGE ring, weights the sync ring, outputs
  gpsimd SWDGE -- three parallel paths.
- ~20 garbage matmuls at kernel start warm the PE clock (HAM) during the
  initial DMA ramp.
"""

import numpy as np
from contextlib import ExitStack

import concourse.bass as bass
import concourse.tile as tile
from concourse import mybir
from concourse.bass_utils import run_bass_kernel_spmd

B, T, D, F, E, TOPK = 2, 1024, 2048, 768, 16, 2
NCORES = 8
EPC = E // NCORES  # experts per core
P = 128


def _split_waits(nc, max_waits=1):
    """walrus on this image rejects >1 sync-wait per instruction
    (setupSyncWait: "Too many sync wait commands"); split extras into
    preceding same-engine NoOps."""
    for f in nc.m.functions:
        for b in f.blocks:
            insts = b.instructions
            idx = 0
            while idx < len(insts):
                inst = insts[idx]
                si = getattr(inst, "sync_info", None)
                if si is not None and si.on_wait and len(si.on_wait) > max_waits:
                    waits = list(si.on_wait)
                    extra, keep = waits[:-max_waits], waits[-max_waits:]
                    pos = idx
                    for j in range(0, len(extra), max_waits):
                        chunk = extra[j : j + max_waits]
                        nop = mybir.InstNoOp(name=f"{inst.name}_ws{j}", ins=[], outs=[])
                        nop.engine = inst.engine
                        nop.sync_info = mybir.SyncInfo(on_wait=chunk, on_update=[])
                        insts.insert(pos, nop)
                        pos += 1
                        idx += 1
                    inst.sync_info = mybir.SyncInfo(
                        on_wait=keep, on_update=list(si.on_update)
                    )
                idx += 1


def build_moe(C):
    """Per-core kernel: EPC experts, capacity C tokens each (C % 8 == 0)."""
    assert C % 8 == 0 and C <= 512
    KD = D // P  # 16 k-tiles over D
    KF = F // P  # 6 f-chunks over F
    MD = D // P  # 16 m-chunks over D (down proj, yT layout)
    XS = 4  # token DMA slabs
    YB = 4  # yT m-chunks batched per output DMA
    bf16 = mybir.dt.bfloat16
    f32 = mybir.dt.float32

    nc = bass.Bass("TRN2", target_bir_lowering=False, debug=False, num_devices=NCORES)
    # host pre-tiled layouts (>=2KB contiguous per partition per DMA):
    #   xgt/xut[e, p, k*C + c] = x_gathered[e, k*128+p, c]   (xut pre-scaled by cw)
    #   wg/wu[e, j, p, k*128+f] = w[e, k*128+p, j*128+f]     (slab per f-chunk j)
    xgt = nc.declare_dram_parameter("xgt", [EPC, P, KD * C], bf16, isOutput=False)
    xut = nc.declare_dram_parameter("xut", [EPC, P, KD * C], bf16, isOutput=False)
    wg = nc.declare_dram_parameter("wg", [EPC, KF, P, KD * P], bf16, isOutput=False)
    wu = nc.declare_dram_parameter("wu", [EPC, KF, P, KD * P], bf16, isOutput=False)
    wd = nc.declare_dram_parameter("wd", [EPC, F, D], bf16, isOutput=False)
    y = nc.declare_dram_parameter("y", [EPC, D, C], bf16, isOutput=True)

    with tile.TileContext(nc) as tc, ExitStack() as ctx:
        xp = ctx.enter_context(tc.tile_pool(name="xp", bufs=2))
        wgp = ctx.enter_context(tc.tile_pool(name="wgp", bufs=2))
        wgp1 = ctx.enter_context(tc.tile_pool(name="wgp1", bufs=1))
        wdp = ctx.enter_context(tc.tile_pool(name="wdp", bufs=2))
        hp = ctx.enter_context(tc.tile_pool(name="hp", bufs=2))
        sp = ctx.enter_context(tc.tile_pool(name="sp", bufs=2))
        op = ctx.enter_context(tc.tile_pool(name="op", bufs=3))
        pg = ctx.enter_context(tc.tile_pool(name="pg", bufs=2, space="PSUM"))
        pu = ctx.enter_context(tc.tile_pool(name="pu", bufs=2, space="PSUM"))
        py = ctx.enter_context(tc.tile_pool(name="py", bufs=4, space="PSUM"))

        # PE warmup: garbage matmuls with no dependencies run during the
        # initial DMA ramp so HAM un-throttles (1.2->2.4GHz) before real work.
        wsb = sp.tile([P, 512], bf16, tag="warm_sb")
        nc.any.memset(wsb[:], 0)
        for _ in range(20):
            wps = py.tile([P, 512], f32, tag="y_ps")
            nc.tensor.matmul(wps[:], wsb[:, :P], wsb[:], start=True, stop=True)

        for e in range(EPC):
            # tokens (gate copy + cw-scaled up copy) on the scalar HW-DGE ring
            xts, uxs = [], []
            for h in range(XS):
                xt = xp.tile([P, (KD // XS) * C], bf16, tag=f"xt{h}")
                nc.scalar.dma_start(xt[:], xgt[e][:, bass.ts(h, (KD // XS) * C)])
                xts.append(xt)
            for h in range(XS):
                ux = xp.tile([P, (KD // XS) * C], bf16, tag=f"ux{h}")
                nc.scalar.dma_start(ux[:], xut[e][:, bass.ts(h, (KD // XS) * C)])
                uxs.append(ux)

            # weights on the sync ring; gate/up slab j=0..2 double-buffered,
            # j=3..5 single-buffered (SBUF headroom; e1 prefetch of the tail
            # slabs just waits for e0's release, which happens early enough).
            gts, uts = [], []
            for j in range(KF):
                pool = wgp if j < 3 else wgp1
                gt = pool.tile([P, KD * P], bf16, tag=f"gt{j}")
                nc.sync.dma_start(gt[:], wg[e, j])
                gts.append(gt)
                ut = pool.tile([P, KD * P], bf16, tag=f"ut{j}")
                nc.sync.dma_start(ut[:], wu[e, j])
                uts.append(ut)

            dts = []
            for h in range(3):
                dt = wdp.tile([P, KF // 3, D], bf16, tag=f"dt{h}")
                nc.sync.dma_start(
                    dt[:],
                    wd[e].rearrange("(k p) d -> p k d", p=P)[:, bass.ts(h, KF // 3), :],
                )
                dts.append(dt)

            # ---- gate/up + SwiGLU -> hT [F, C] bf16 ----
            ht = hp.tile([P, KF, C], bf16, tag="ht")
            for j in range(KF):
                g_ps = pg.tile([P, C], f32, tag="g_ps")
                u_ps = pu.tile([P, C], f32, tag="u_ps")
                for k in range(KD):
                    nc.tensor.matmul(
                        g_ps[:],
                        gts[j][:, bass.ts(k, P)],
                        xts[k // (KD // XS)][:, bass.ts(k % (KD // XS), C)],
                        start=(k == 0),
                        stop=(k == KD - 1),
                    )
                for k in range(KD):
                    nc.tensor.matmul(
                        u_ps[:],
                        uts[j][:, bass.ts(k, P)],
                        uxs[k // (KD // XS)][:, bass.ts(k % (KD // XS), C)],
                        start=(k == 0),
                        stop=(k == KD - 1),
                    )
                sil = sp.tile([P, C], f32, tag="sil")
                nc.scalar.activation(
                    sil[:], g_ps[:], mybir.ActivationFunctionType.Silu
                )
                nc.vector.tensor_mul(ht[:, j, :], sil[:], u_ps[:])

            # ---- down proj: yT[m-chunk, :] = sum_j wd[j,m].T @ hT[j] ----
            ydst = y[e].rearrange("(m p) c -> p m c", p=P)
            for m0 in range(0, MD, YB):
                ysb = op.tile([P, YB, C], bf16, tag="ysb")
                for mi in range(YB):
                    m = m0 + mi
                    y_ps = py.tile([P, C], f32, tag="y_ps")
                    for j in range(KF):
                        nc.tensor.matmul(
                            y_ps[:],
                            dts[j // (KF // 3)][:, j % (KF // 3), bass.ts(m, P)],
                            ht[:, j, :],
                            start=(j == 0),
                            stop=(j == KF - 1),
                        )
                    nc.vector.tensor_copy(ysb[:, mi, :], y_ps[:])
                nc.gpsimd.dma_start(ydst[:, m0 : m0 + YB, :], ysb[:])

    _split_waits(nc)
    return nc


_CACHE = {}


def _get_nc(C):
    if C not in _CACHE:
        _CACHE[C] = build_moe(C)
    return _CACHE[C]


def _route(x, router_w):
    """Replicates the reference router in f32: softmax over expert scores,
    top-2, renormalize."""
    xf = x.reshape(-1, D).astype(np.float32)
    scores = xf @ router_w.astype(np.float32)
    m = scores.max(axis=-1, keepdims=True)
    ex = np.exp(scores - m)
    probs = ex / ex.sum(axis=-1, keepdims=True)
    idx = np.argsort(-probs, axis=-1, kind="stable")[:, :TOPK]
    wts = np.take_along_axis(probs, idx, axis=-1)
    wts = wts / wts.sum(axis=-1, keepdims=True)
    return idx.astype(np.int32), wts.astype(np.float32)


def kernel(x, router_w, gate_w, up_w, down_w):
    import ml_dtypes

    bf = ml_dtypes.bfloat16

    x = np.asarray(x)
    in_dtype = x.dtype
    xf = x.reshape(-1, D).astype(np.float32)
    idx, wts = _route(x, np.asarray(router_w))

    # token lists per expert
    tok_ids = [None] * E
    tok_wts = [None] * E
    for e in range(E):
        sel = np.nonzero(idx == e)
        tok_ids[e] = sel[0].astype(np.int64)
        tok_wts[e] = wts[sel[0], sel[1]]
    max_n = max(len(t) for t in tok_ids)
    C = min(512, max(P, -(-max_n // 8) * 8))

    nc = _get_nc(C)

    KD, KF = D // P, F // P

    def tile_gateup(w):
        # [E, D, F] -> [E, KF, P, KD*P] with w_t[e,j,p,k*P+f] = w[e,k*P+p,j*P+f]
        w = np.asarray(w).astype(bf)
        w = w.reshape(E, KD, P, KF, P).transpose(0, 3, 2, 1, 4)
        return np.ascontiguousarray(w.reshape(E, KF, P, KD * P))

    g16 = tile_gateup(gate_w)
    u16 = tile_gateup(up_w)
    d16 = np.asarray(down_w).astype(bf)
    xT = np.ascontiguousarray(xf.T)  # [D, B*T] f32

    in_maps = []
    for c in range(NCORES):
        xg = np.zeros((EPC, P, KD, C), dtype=bf)
        xu = np.zeros((EPC, P, KD, C), dtype=bf)
        for j in range(EPC):
            e = c * EPC + j
            n = len(tok_ids[e])
            gath = xT[:, tok_ids[e]]  # [D, n] f32
            xg[j, :, :, :n] = gath.astype(bf).reshape(KD, P, n).transpose(1, 0, 2)
            xu[j, :, :, :n] = (
                (gath * tok_wts[e][None, :])
                .astype(bf)
                .reshape(KD, P, n)
                .transpose(1, 0, 2)
            )
        in_maps.append(
            {
                "xgt": xg.reshape(EPC, P, KD * C),
                "xut": xu.reshape(EPC, P, KD * C),
                "wg": np.ascontiguousarray(g16[c * EPC : (c + 1) * EPC]),
                "wu": np.ascontiguousarray(u16[c * EPC : (c + 1) * EPC]),
                "wd": np.ascontiguousarray(d16[c * EPC : (c + 1) * EPC]),
            }
        )

    res = run_bass_kernel_spmd(nc, in_maps, list(range(NCORES)))

    out = np.zeros((B * T, D), dtype=np.float32)
    for c in range(NCORES):
        yv = res.results[c]["y"]  # [EPC, D, C] bf16
        for j in range(EPC):
            e = c * EPC + j
            n = len(tok_ids[e])
            out[tok_ids[e]] += yv[j, :, :n].astype(np.float32).T
    return out.reshape(B, T, D).astype(in_dtype)


# revision 24
# speedup vs baseline: 1.1121x; 1.1121x over previous
"""MoE layer (B=2,T=1024,D=2048,F=768,E=16,K=2) on 8 NeuronCores.

Expert-parallel: core c owns experts {2c, 2c+1}. Host computes the router
(scores -> softmax -> top-2 -> renormalize; ~0.3% of total FLOPs), gathers
each expert's tokens into fixed-capacity transposed buffers, and the device
kernel runs the sparse SwiGLU FFN (gate/up/down matmuls) in bf16 with f32
PSUM accumulation.

Layout tricks:
- tokens staged transposed (xgt [D, C]) so gate/up produce hT [F, C] directly
  in the lhsT layout the down projection wants -- zero on-device transposes.
- the routing weight is pre-multiplied into the up-projection copy of the
  tokens (h = silu(g) * (u * cw)), so outputs come out pre-scaled.
- down projection emits yT [D, C]: matmul free dim = C exactly, no
  padded-token chunk rounding; host transposes during the scatter-add.
- weights host-pre-tiled so every DMA is >=4KB-contiguous per partition;
  token DMAs ride the scalar HW-DGE ring, weights the sync ring, outputs
  gpsimd SWDGE -- three parallel paths.
- ~20 garbage matmuls at kernel start warm the PE clock (HAM) during the
  initial DMA ramp.
"""

import numpy as np
from contextlib import ExitStack

import concourse.bass as bass
import concourse.tile as tile
from concourse import mybir
from concourse.bass_utils import run_bass_kernel_spmd

B, T, D, F, E, TOPK = 2, 1024, 2048, 768, 16, 2
NCORES = 8
EPC = E // NCORES  # experts per core
P = 128


def _split_waits(nc, max_waits=1):
    """walrus on this image rejects >1 sync-wait per instruction
    (setupSyncWait: "Too many sync wait commands"); split extras into
    preceding same-engine NoOps."""
    for f in nc.m.functions:
        for b in f.blocks:
            insts = b.instructions
            idx = 0
            while idx < len(insts):
                inst = insts[idx]
                si = getattr(inst, "sync_info", None)
                if si is not None and si.on_wait and len(si.on_wait) > max_waits:
                    waits = list(si.on_wait)
                    extra, keep = waits[:-max_waits], waits[-max_waits:]
                    pos = idx
                    for j in range(0, len(extra), max_waits):
                        chunk = extra[j : j + max_waits]
                        nop = mybir.InstNoOp(name=f"{inst.name}_ws{j}", ins=[], outs=[])
                        nop.engine = inst.engine
                        nop.sync_info = mybir.SyncInfo(on_wait=chunk, on_update=[])
                        insts.insert(pos, nop)
                        pos += 1
                        idx += 1
                    inst.sync_info = mybir.SyncInfo(
                        on_wait=keep, on_update=list(si.on_update)
                    )
                idx += 1


def build_moe(C):
    """Per-core kernel: EPC experts, capacity C tokens each (C % 8 == 0)."""
    assert C % 8 == 0 and C <= 512
    KD = D // P  # 16 k-tiles over D
    KF = F // P  # 6 f-chunks over F
    MD = D // P  # 16 m-chunks over D (down proj, yT layout)
    XS = 4  # token DMA slabs
    YB = 4  # yT m-chunks batched per output DMA
    bf16 = mybir.dt.bfloat16
    f32 = mybir.dt.float32

    nc = bass.Bass("TRN2", target_bir_lowering=False, debug=False, num_devices=NCORES)
    # host pre-tiled layouts (>=2KB contiguous per partition per DMA):
    #   xgt/xut[e, p, k*C + c] = x_gathered[e, k*128+p, c]   (xut pre-scaled by cw)
    #   wg/wu[e, j, p, k*128+f] = w[e, k*128+p, j*128+f]     (slab per f-chunk j)
    xgt = nc.declare_dram_parameter("xgt", [EPC, P, KD * C], bf16, isOutput=False)
    cw = nc.declare_dram_parameter("cw", [EPC, P, C], f32, isOutput=False)
    wg = nc.declare_dram_parameter("wg", [EPC, KF, P, KD * P], bf16, isOutput=False)
    wu = nc.declare_dram_parameter("wu", [EPC, KF, P, KD * P], bf16, isOutput=False)
    wd = nc.declare_dram_parameter("wd", [EPC, F, D], bf16, isOutput=False)
    y = nc.declare_dram_parameter("y", [EPC, D, C], bf16, isOutput=True)

    with tile.TileContext(nc) as tc, ExitStack() as ctx:
        xp = ctx.enter_context(tc.tile_pool(name="xp", bufs=2))
        wgp = ctx.enter_context(tc.tile_pool(name="wgp", bufs=2))
        wgp1 = ctx.enter_context(tc.tile_pool(name="wgp1", bufs=1))
        wdp = ctx.enter_context(tc.tile_pool(name="wdp", bufs=2))
        hp = ctx.enter_context(tc.tile_pool(name="hp", bufs=2))
        sp = ctx.enter_context(tc.tile_pool(name="sp", bufs=2))
        op = ctx.enter_context(tc.tile_pool(name="op", bufs=3))
        pg = ctx.enter_context(tc.tile_pool(name="pg", bufs=2, space="PSUM"))
        pu = ctx.enter_context(tc.tile_pool(name="pu", bufs=2, space="PSUM"))
        py = ctx.enter_context(tc.tile_pool(name="py", bufs=4, space="PSUM"))

        # PE warmup: garbage matmuls with no dependencies run during the
        # initial DMA ramp so HAM un-throttles (1.2->2.4GHz) before real work.
        wsb = sp.tile([P, 512], bf16, tag="warm_sb")
        nc.any.memset(wsb[:], 0)
        for _ in range(16):
            wps = py.tile([P, 512], f32, tag="y_ps")
            nc.tensor.matmul(wps[:], wsb[:, :P], wsb[:], start=True, stop=True)

        for e in range(EPC):
            # tokens on the scalar HW-DGE ring (small, needed first)
            xts = []
            for h in range(XS):
                xt = xp.tile([P, (KD // XS) * C], bf16, tag=f"xt{h}")
                nc.scalar.dma_start(xt[:], xgt[e][:, bass.ts(h, (KD // XS) * C)])
                xts.append(xt)

            # routing weights, host-replicated across partitions; fused into
            # the down-proj PSUM eviction multiply.
            cwb = sp.tile([P, C], f32, tag="cwb")
            nc.gpsimd.dma_start(cwb[:], cw[e])

            # weights on the sync ring; gate/up slab j=0..2 double-buffered,
            # j=3..5 single-buffered (SBUF headroom; e1 prefetch of the tail
            # slabs just waits for e0's release, which happens early enough).
            gts, uts = [], []
            for j in range(KF):
                pool = wgp if j < 3 else wgp1
                gt = pool.tile([P, KD * P], bf16, tag=f"gt{j}")
                nc.sync.dma_start(gt[:], wg[e, j])
                gts.append(gt)
                ut = pool.tile([P, KD * P], bf16, tag=f"ut{j}")
                nc.sync.dma_start(ut[:], wu[e, j])
                uts.append(ut)

            dts = []
            for h in range(3):
                dt = wdp.tile([P, KF // 3, D], bf16, tag=f"dt{h}")
                nc.sync.dma_start(
                    dt[:],
                    wd[e].rearrange("(k p) d -> p k d", p=P)[:, bass.ts(h, KF // 3), :],
                )
                dts.append(dt)

            # ---- gate/up + SwiGLU -> hT [F, C] bf16 ----
            ht = hp.tile([P, KF, C], bf16, tag="ht")
            for j in range(KF):
                g_ps = pg.tile([P, C], f32, tag="g_ps")
                u_ps = pu.tile([P, C], f32, tag="u_ps")
                for k in range(KD):
                    nc.tensor.matmul(
                        g_ps[:],
                        gts[j][:, bass.ts(k, P)],
                        xts[k // (KD // XS)][:, bass.ts(k % (KD // XS), C)],
                        start=(k == 0),
                        stop=(k == KD - 1),
                    )
                for k in range(KD):
                    nc.tensor.matmul(
                        u_ps[:],
                        uts[j][:, bass.ts(k, P)],
                        xts[k // (KD // XS)][:, bass.ts(k % (KD // XS), C)],
                        start=(k == 0),
                        stop=(k == KD - 1),
                    )
                sil = sp.tile([P, C], f32, tag="sil")
                nc.scalar.activation(
                    sil[:], g_ps[:], mybir.ActivationFunctionType.Silu
                )
                nc.vector.tensor_mul(ht[:, j, :], sil[:], u_ps[:])

            # ---- down proj: yT[m-chunk, :] = sum_j wd[j,m].T @ hT[j] ----
            ydst = y[e].rearrange("(m p) c -> p m c", p=P)
            # last expert's outputs ride the (idle by then) scalar HW-DGE ring
            # for a shorter completion tail than gpsimd SWDGE.
            yeng = nc.scalar if e == EPC - 1 else nc.gpsimd
            for m0 in range(0, MD, YB):
                ysb = op.tile([P, YB, C], bf16, tag="ysb")
                for mi in range(YB):
                    m = m0 + mi
                    y_ps = py.tile([P, C], f32, tag="y_ps")
                    for j in range(KF):
                        nc.tensor.matmul(
                            y_ps[:],
                            dts[j // (KF // 3)][:, j % (KF // 3), bass.ts(m, P)],
                            ht[:, j, :],
                            start=(j == 0),
                            stop=(j == KF - 1),
                        )
                    nc.vector.tensor_mul(ysb[:, mi, :], y_ps[:], cwb[:])
                yeng.dma_start(ydst[:, m0 : m0 + YB, :], ysb[:])

    _split_waits(nc)
    return nc


_CACHE = {}


def _get_nc(C):
    if C not in _CACHE:
        _CACHE[C] = build_moe(C)
    return _CACHE[C]


def _route(x, router_w):
    """Replicates the reference router in f32: softmax over expert scores,
    top-2, renormalize."""
    xf = x.reshape(-1, D).astype(np.float32)
    scores = xf @ router_w.astype(np.float32)
    m = scores.max(axis=-1, keepdims=True)
    ex = np.exp(scores - m)
    probs = ex / ex.sum(axis=-1, keepdims=True)
    idx = np.argsort(-probs, axis=-1, kind="stable")[:, :TOPK]
    wts = np.take_along_axis(probs, idx, axis=-1)
    wts = wts / wts.sum(axis=-1, keepdims=True)
    return idx.astype(np.int32), wts.astype(np.float32)


def kernel(x, router_w, gate_w, up_w, down_w):
    import ml_dtypes

    bf = ml_dtypes.bfloat16

    x = np.asarray(x)
    in_dtype = x.dtype
    xf = x.reshape(-1, D).astype(np.float32)
    idx, wts = _route(x, np.asarray(router_w))

    # token lists per expert
    tok_ids = [None] * E
    tok_wts = [None] * E
    for e in range(E):
        sel = np.nonzero(idx == e)
        tok_ids[e] = sel[0].astype(np.int64)
        tok_wts[e] = wts[sel[0], sel[1]]
    max_n = max(len(t) for t in tok_ids)
    C = min(512, max(P, -(-max_n // 8) * 8))

    nc = _get_nc(C)

    KD, KF = D // P, F // P

    def tile_gateup(w):
        # [E, D, F] -> [E, KF, P, KD*P] with w_t[e,j,p,k*P+f] = w[e,k*P+p,j*P+f]
        w = np.asarray(w).astype(bf)
        w = w.reshape(E, KD, P, KF, P).transpose(0, 3, 2, 1, 4)
        return np.ascontiguousarray(w.reshape(E, KF, P, KD * P))

    g16 = tile_gateup(gate_w)
    u16 = tile_gateup(up_w)
    d16 = np.asarray(down_w).astype(bf)
    xT = np.ascontiguousarray(xf.T)  # [D, B*T] f32

    in_maps = []
    for c in range(NCORES):
        xg = np.zeros((EPC, P, KD, C), dtype=bf)
        cwv = np.zeros((EPC, P, C), dtype=np.float32)
        for j in range(EPC):
            e = c * EPC + j
            n = len(tok_ids[e])
            gath = xT[:, tok_ids[e]]  # [D, n] f32
            xg[j, :, :, :n] = gath.astype(bf).reshape(KD, P, n).transpose(1, 0, 2)
            cwv[j, :, :n] = tok_wts[e][None, :]
        in_maps.append(
            {
                "xgt": xg.reshape(EPC, P, KD * C),
                "cw": cwv,
                "wg": np.ascontiguousarray(g16[c * EPC : (c + 1) * EPC]),
                "wu": np.ascontiguousarray(u16[c * EPC : (c + 1) * EPC]),
                "wd": np.ascontiguousarray(d16[c * EPC : (c + 1) * EPC]),
            }
        )

    res = run_bass_kernel_spmd(nc, in_maps, list(range(NCORES)))

    out = np.zeros((B * T, D), dtype=np.float32)
    for c in range(NCORES):
        yv = res.results[c]["y"]  # [EPC, D, C] bf16
        for j in range(EPC):
            e = c * EPC + j
            n = len(tok_ids[e])
            out[tok_ids[e]] += yv[j, :, :n].astype(np.float32).T
    return out.reshape(B, T, D).astype(in_dtype)
